# revision 10
# baseline (speedup 1.0000x reference)
"""Bahdanau additive-attention kernel for Trainium2, SPMD over 8 NeuronCores.

Reference computation (per batch b):
    dec_t  = dec @ W                                  [TD, D]
    score  = sum_d V[d] * tanh(dec_t[td,d] + enc[te,d])   [TD, TE]
    attn   = softmax(score, axis=te)
    ctx    = attn @ enc                               [TD, D]

Sharding: B=4, 8 cores -> core c handles batch b = c//2, td half h = c%2
(256 td rows each); enc/W replicated per batch. Host side does layout
marshalling only (transposes / dtype casts / placing V on diagonals);
all FLOPs of the reference computation run on device.

Per-core pipeline (sizes hardcoded: TD_N=256 td rows, TE=512, D=128):
 - PE:  dec_tT[e, td] = W.T-contracted matmul on the device.
 - DVE: tadd[d, te] = encT_bf[d, te] + dec_tT[d, td] via tensor_scalar_add,
   bf16 in/out -> 4x DVE mode (~345 ns per td row).
 - ACT: tanh over big batched instructions (bf16). This is the roofline
   engine: 16.8M elems / 128 lanes / 1.2 GHz ~ 110 us busy.
 - PE:  score row for td via accumulating matmul, lhsT = [128, 32] variant
   with V embedded in column m (v_bank). Four matmuls targeting the four
   32-partition column strips of the PSUM score tile run concurrently
   (tile_position col tiling); permutation td = 4*m + g lives at PSUM
   partition 32*g + m and is undone by the output DMA.
 - softmax without max-subtraction (|score| <= ||V||_1 ~ 14, exp safe):
   ACT exp(PSUM)->bf16, PE transpose, context matmul against [enc | ones]
   giving unnormalized context + denominator, then DVE reciprocal+scale.
"""

from contextlib import ExitStack

import numpy as np

import concourse.bacc as bacc
import concourse.tile as tile
from concourse import mybir
from concourse.bass_utils import run_bass_kernel_spmd

F32 = mybir.dt.float32
BF16 = mybir.dt.bfloat16

B, TD, TE, D = 4, 512, 512, 128
N_CORES = 8
TD_N = (B * TD) // N_CORES          # 256 td rows per core
P = 128
STRIP = 32                          # col-tiling strip width
N_STRIP = P // STRIP                # 4
# per-128-row-block ACT batch schedules: small first batches hide the DVE
# ramp; small last batches let the final score matmuls drain immediately.
BATCHES_FIRST = [4, 4, 8, 16, 32, 64]
BATCHES_LAST = [64, 32, 16, 8, 4, 4]


def _build_body(ctx, tc, out_ap, decT_ap, encT_ap, enc_ones_ap, v_bank_ap,
                w_ap, ident_ap, td_n):
    nc = tc.nc
    n_blk = td_n // P
    n_te_chunk = TE // P

    consts = ctx.enter_context(tc.tile_pool(name="consts", bufs=1))
    setup_ps = ctx.enter_context(tc.tile_pool(name="setup_ps", bufs=1, space="PSUM"))
    tadd_pool = ctx.enter_context(tc.tile_pool(name="tadd", bufs=2))
    score_ps_pool = ctx.enter_context(tc.tile_pool(name="score_ps", bufs=2, space="PSUM"))
    t_ps_pool = ctx.enter_context(tc.tile_pool(name="t_ps", bufs=2, space="PSUM"))
    ctx_ps_pool = ctx.enter_context(tc.tile_pool(name="ctx_ps", bufs=2, space="PSUM"))
    esc_pool = ctx.enter_context(tc.tile_pool(name="esc", bufs=2))
    out_pool = ctx.enter_context(tc.tile_pool(name="outp", bufs=2))

    # ---- inputs (pre-marshalled on host); chunked so the first adds start
    # as early as possible ----
    decT = consts.tile([P, td_n], F32)            # [d, td]
    nc.sync.dma_start(out=decT[:, 0:P], in_=decT_ap[:, 0:P])
    w_sb = consts.tile([P, P], F32)               # [d, e]
    nc.sync.dma_start(out=w_sb, in_=w_ap)
    encT_bf = consts.tile([P, TE], BF16)          # [d, te]
    nc.sync.dma_start(out=encT_bf, in_=encT_ap)
    for c in range(1, n_blk):
        nc.sync.dma_start(
            out=decT[:, c * P : (c + 1) * P], in_=decT_ap[:, c * P : (c + 1) * P]
        )
    v_bank = consts.tile([P, STRIP, STRIP], BF16)  # variant m: V in col m
    nc.sync.dma_start(out=v_bank, in_=v_bank_ap)
    enc_ones = consts.tile([P, n_te_chunk, P + 1], BF16)   # [te | 1.0]
    nc.sync.dma_start(out=enc_ones, in_=enc_ones_ap)
    ident_bf = consts.tile([P, P], BF16)
    nc.sync.dma_start(out=ident_bf, in_=ident_ap)

    # dec_tT[e, td] = sum_d W[d,e] * decT[d, td]; one tile per block so the
    # first adds only wait on chunk 0.
    dec_tT = []
    for c in range(n_blk):
        mp = setup_ps.tile([P, P], F32)
        nc.tensor.matmul(mp, w_sb, decT[:, c * P : (c + 1) * P], start=True, stop=True)
        t = consts.tile([P, P], F32, tag=f"dec_tT{c}")
        nc.vector.tensor_copy(t, mp)
        dec_tT.append(t)

    # ---- main loop ----
    for blk in range(n_blk):
        batches = BATCHES_FIRST if blk == 0 else BATCHES_LAST
        assert sum(batches) == P
        score_ps = score_ps_pool.tile([P, TE], F32)
        j0 = 0
        for bs in batches:
            tadd = tadd_pool.tile([P, bs, TE], BF16, tag="tadd")
            for k in range(bs):
                nc.vector.tensor_scalar_add(
                    out=tadd[:, k, :], in0=encT_bf,
                    scalar1=dec_tT[blk][:, j0 + k : j0 + k + 1],
                )
            # tanh in place: halves SBUF traffic and tile count
            tanh_bf = tadd
            nc.scalar.activation(
                out=tanh_bf, in_=tadd, func=mybir.ActivationFunctionType.Tanh
            )
            # V-reduce: td-in-block j = 4*m + g -> PSUM partition 32*g + m.
            # Quad of col-strip matmuls per m runs concurrently on PE.
            for k in range(bs):
                j = j0 + k
                m, g = divmod(j, N_STRIP)
                nc.tensor.matmul(
                    score_ps[g * STRIP : (g + 1) * STRIP, :],
                    v_bank[:, m, :],
                    tanh_bf[:, k, :],
                    start=(m == 0),
                    stop=(m == STRIP - 1),
                    tile_position=(0, g * STRIP),
                    skip_group_check=True,
                )
            j0 += bs

        # ---- epilogue for this block (rows are permuted) ----
        last = blk == n_blk - 1
        ctx_ps = ctx_ps_pool.tile([P, P + 1], F32)
        escore = esc_pool.tile([P, TE], BF16, tag="escore")
        tps = t_ps_pool.tile([P, n_te_chunk, P], BF16)
        escT = esc_pool.tile([P, n_te_chunk, P], BF16, tag="escT")
        if not last:
            # off the critical path: one big exp, then transposes
            nc.scalar.activation(
                out=escore, in_=score_ps, func=mybir.ActivationFunctionType.Exp
            )
            for c in range(n_te_chunk):
                nc.tensor.transpose(
                    tps[:, c, :], escore[:, c * P : (c + 1) * P], ident_bf
                )
            nc.vector.tensor_copy(escT, tps)
            for c in range(n_te_chunk):
                nc.tensor.matmul(
                    ctx_ps, escT[:, c, :], enc_ones[:, c, :],
                    start=(c == 0), stop=(c == n_te_chunk - 1),
                )
        else:
            # tail-latency critical: pipeline exp/transpose/copy/matmul per chunk
            for c in range(n_te_chunk):
                nc.scalar.activation(
                    out=escore[:, c * P : (c + 1) * P],
                    in_=score_ps[:, c * P : (c + 1) * P],
                    func=mybir.ActivationFunctionType.Exp,
                )
                nc.tensor.transpose(
                    tps[:, c, :], escore[:, c * P : (c + 1) * P], ident_bf
                )
                nc.vector.tensor_copy(escT[:, c, :], tps[:, c, :])
                nc.tensor.matmul(
                    ctx_ps, escT[:, c, :], enc_ones[:, c, :],
                    start=(c == 0), stop=(c == n_te_chunk - 1),
                )
        recip = out_pool.tile([P, 1], F32, tag="recip")
        nc.vector.reciprocal(recip, ctx_ps[:, P : P + 1])
        ctx_sb = out_pool.tile([P, P], F32, tag="ctx_sb")
        nc.vector.tensor_scalar_mul(out=ctx_sb, in0=ctx_ps[:, 0:P], scalar1=recip)

        # un-permute rows: PSUM partition 32g+m holds td 4m+g
        blk_rows = out_ap[blk * P : (blk + 1) * P, :].rearrange(
            "(m four) d -> four m d", four=N_STRIP
        )
        for g in range(N_STRIP):
            nc.sync.dma_start(
                out=blk_rows[g], in_=ctx_sb[g * STRIP : (g + 1) * STRIP, :]
            )


def build_program(td_n=TD_N):
    nc = bacc.Bacc("TRN2", target_bir_lowering=False, debug=False)
    n_te_chunk = TE // P
    decT = nc.dram_tensor("decT", [P, td_n], F32, kind="ExternalInput").ap()
    encT = nc.dram_tensor("encT", [P, TE], BF16, kind="ExternalInput").ap()
    enc_ones = nc.dram_tensor(
        "enc_ones", [P, n_te_chunk, P + 1], BF16, kind="ExternalInput"
    ).ap()
    v_bank = nc.dram_tensor("v_bank", [P, STRIP, STRIP], BF16, kind="ExternalInput").ap()
    w = nc.dram_tensor("w", [D, D], F32, kind="ExternalInput").ap()
    ident = nc.dram_tensor("ident", [P, P], BF16, kind="ExternalInput").ap()
    out = nc.dram_tensor("ctx_out", [td_n, D], F32, kind="ExternalOutput").ap()
    with tile.TileContext(nc) as tc, ExitStack() as ctx:
        _build_body(ctx, tc, out, decT, encT, enc_ones, v_bank, w, ident, td_n)
    nc.compile()
    return nc


def _prep_core_inputs(dec_slice, enc_b, w, v, bf16):
    """Host-side layout marshalling for one core (no reference FLOPs)."""
    n_te_chunk = TE // P
    decT = np.ascontiguousarray(dec_slice.T)                       # [d, td]
    encT = np.ascontiguousarray(enc_b.T).astype(bf16)              # [d, te]
    enc_ones = np.ones((P, n_te_chunk, P + 1), dtype=np.float32)
    # enc_ones[p, c, 0:128] = enc[c*128 + p, :]
    enc_ones[:, :, :P] = enc_b.reshape(n_te_chunk, P, D).transpose(1, 0, 2)
    v_bank = np.zeros((P, STRIP, STRIP), dtype=np.float32)
    idx = np.arange(STRIP)
    v_bank[:, idx, idx] = v[:, 0:1] * np.ones((P, STRIP), dtype=np.float32)
    ident = np.eye(P, dtype=np.float32)
    return {
        "decT": decT,
        "encT": encT,
        "enc_ones": enc_ones.astype(bf16),
        "v_bank": v_bank.astype(bf16),
        "w": np.ascontiguousarray(w),
        "ident": ident.astype(bf16),
    }


_CACHED_NC = None


def _run(inputs, trace=False):
    global _CACHED_NC
    if _CACHED_NC is None:
        _CACHED_NC = build_program()
    nc = _CACHED_NC
    bf16 = mybir.dt.np(BF16)

    dec = np.ascontiguousarray(inputs["decoder_outputs"], dtype=np.float32)
    enc = np.ascontiguousarray(inputs["encoder_outputs"], dtype=np.float32)
    w = np.ascontiguousarray(inputs["W"], dtype=np.float32)
    v = np.ascontiguousarray(inputs["V"], dtype=np.float32)

    in_maps = []
    for c in range(N_CORES):
        b, h = divmod(c, 2)
        in_maps.append(
            _prep_core_inputs(dec[b, h * TD_N : (h + 1) * TD_N], enc[b], w, v, bf16)
        )
    res = run_bass_kernel_spmd(nc, in_maps, core_ids=list(range(N_CORES)), trace=trace)
    out = np.zeros((B, TD, D), dtype=np.float32)
    for c in range(N_CORES):
        b, h = divmod(c, 2)
        out[b, h * TD_N : (h + 1) * TD_N] = res.results[c]["ctx_out"]
    return out, res


def kernel(**inputs):
    out, _ = _run(inputs, trace=False)
    return out


if __name__ == "__main__":
    rng = np.random.default_rng(0)
    inputs = {
        "decoder_outputs": rng.standard_normal((B, TD, D)).astype(np.float32),
        "encoder_outputs": rng.standard_normal((B, TE, D)).astype(np.float32),
        "W": (rng.uniform(-0.15, 0.15, (D, D))).astype(np.float32),
        "V": (rng.uniform(-0.21, 0.21, (D, 1))).astype(np.float32),
    }
    out = kernel(**inputs)
    print("ran, output shape", out.shape)


# revision 14
# speedup vs baseline: 1.0499x; 1.0499x over previous
"""Bahdanau additive-attention kernel for Trainium2, SPMD over 8 NeuronCores.

Reference computation (per batch b):
    dec_t  = dec @ W                                  [TD, D]
    score  = sum_d V[d] * tanh(dec_t[td,d] + enc[te,d])   [TD, TE]
    attn   = softmax(score, axis=te)
    ctx    = attn @ enc                               [TD, D]

Sharding: B=4, 8 cores -> core c handles batch b = c//2, td half h = c%2
(256 td rows each); enc/W replicated per batch. Host side does layout
marshalling only (transposes / dtype casts / placing V on diagonals);
all FLOPs of the reference computation run on device.

Per-core pipeline (sizes hardcoded: TD_N=256 td rows, TE=512, D=128):
 - PE:  dec_tT[e, td] = W.T-contracted matmul on the device.
 - DVE: tadd[d, te] = encT_bf[d, te] + dec_tT[d, td] via tensor_scalar_add,
   bf16 in/out -> 4x DVE mode (~345 ns per td row).
 - ACT: tanh over big batched instructions (bf16). This is the roofline
   engine: 16.8M elems / 128 lanes / 1.2 GHz ~ 110 us busy.
 - PE:  score row for td via accumulating matmul, lhsT = [128, 32] variant
   with V embedded in column m (v_bank). Four matmuls targeting the four
   32-partition column strips of the PSUM score tile run concurrently
   (tile_position col tiling); permutation td = 4*m + g lives at PSUM
   partition 32*g + m and is undone by the output DMA.
 - softmax without max-subtraction (|score| <= ||V||_1 ~ 14, exp safe):
   ACT exp(PSUM)->bf16, PE transpose, context matmul against [enc | ones]
   giving unnormalized context + denominator, then DVE reciprocal+scale.
"""

from contextlib import ExitStack

import numpy as np

import concourse.bacc as bacc
import concourse.tile as tile
from concourse import mybir
from concourse.bass_utils import run_bass_kernel_spmd

F32 = mybir.dt.float32
BF16 = mybir.dt.bfloat16

B, TD, TE, D = 4, 512, 512, 128
N_CORES = 8
TD_N = (B * TD) // N_CORES          # 256 td rows per core
P = 128
STRIP = 32                          # col-tiling strip width
N_STRIP = P // STRIP                # 4
# per-128-row-block ACT batch schedules: small first batches hide the DVE
# ramp; small last batches let the final score matmuls drain immediately.
BATCHES_FIRST = [4, 4, 8, 16, 32, 32, 32]
BATCHES_LAST = [32, 32, 32, 16, 8, 4, 4]


def _build_body(ctx, tc, out_ap, decT_ap, encT_ap, enc_ones_ap, v_bank_ap,
                w_ap, ident_ap, td_n):
    nc = tc.nc
    n_blk = td_n // P
    n_te_chunk = TE // P

    consts = ctx.enter_context(tc.tile_pool(name="consts", bufs=1))
    setup_ps = ctx.enter_context(tc.tile_pool(name="setup_ps", bufs=1, space="PSUM"))
    tadd_pool = ctx.enter_context(tc.tile_pool(name="tadd", bufs=3))
    score_ps_pool = ctx.enter_context(tc.tile_pool(name="score_ps", bufs=2, space="PSUM"))
    t_ps_pool = ctx.enter_context(tc.tile_pool(name="t_ps", bufs=2, space="PSUM"))
    ctx_ps_pool = ctx.enter_context(tc.tile_pool(name="ctx_ps", bufs=2, space="PSUM"))
    esc_pool = ctx.enter_context(tc.tile_pool(name="esc", bufs=2))
    out_pool = ctx.enter_context(tc.tile_pool(name="outp", bufs=2))

    # ---- inputs (pre-marshalled on host); chunked so the first adds start
    # as early as possible ----
    decT = consts.tile([P, td_n], F32)            # [d, td]
    nc.sync.dma_start(out=decT[:, 0:P], in_=decT_ap[:, 0:P])
    w_sb = consts.tile([P, P], F32)               # [d, e]
    nc.scalar.dma_start(out=w_sb, in_=w_ap)
    encT_bf = consts.tile([P, TE], BF16)          # [d, te]
    nc.sync.dma_start(out=encT_bf, in_=encT_ap)
    for c in range(1, n_blk):
        nc.scalar.dma_start(
            out=decT[:, c * P : (c + 1) * P], in_=decT_ap[:, c * P : (c + 1) * P]
        )
    v_bank = consts.tile([P, STRIP, STRIP], BF16)  # variant m: V in col m
    nc.gpsimd.dma_start(out=v_bank, in_=v_bank_ap)
    enc_ones = consts.tile([P, n_te_chunk, P + 1], BF16)   # [te | 1.0]
    nc.gpsimd.dma_start(out=enc_ones, in_=enc_ones_ap)
    ident_bf = consts.tile([P, P], BF16)
    nc.gpsimd.dma_start(out=ident_bf, in_=ident_ap)

    # dec_tT[e, td] = sum_d W[d,e] * decT[d, td]; one tile per block so the
    # first adds only wait on chunk 0.
    dec_tT = []
    for c in range(n_blk):
        mp = setup_ps.tile([P, P], F32)
        nc.tensor.matmul(mp, w_sb, decT[:, c * P : (c + 1) * P], start=True, stop=True)
        t = consts.tile([P, P], F32, tag=f"dec_tT{c}")
        nc.vector.tensor_copy(t, mp)
        dec_tT.append(t)

    # ---- main loop ----
    for blk in range(n_blk):
        batches = BATCHES_FIRST if blk == 0 else BATCHES_LAST
        assert sum(batches) == P
        score_ps = score_ps_pool.tile([P, TE], F32)
        j0 = 0
        for bs in batches:
            tadd = tadd_pool.tile([P, bs, TE], BF16, tag="tadd")
            for k in range(bs):
                nc.vector.tensor_scalar_add(
                    out=tadd[:, k, :], in0=encT_bf,
                    scalar1=dec_tT[blk][:, j0 + k : j0 + k + 1],
                )
            # tanh in place: halves SBUF traffic and tile count
            tanh_bf = tadd
            nc.scalar.activation(
                out=tanh_bf, in_=tadd, func=mybir.ActivationFunctionType.Tanh
            )
            # V-reduce: td-in-block j = 4*m + g -> PSUM partition 32*g + m.
            # Quad of col-strip matmuls per m runs concurrently on PE.
            for k in range(bs):
                j = j0 + k
                m, g = divmod(j, N_STRIP)
                nc.tensor.matmul(
                    score_ps[g * STRIP : (g + 1) * STRIP, :],
                    v_bank[:, m, :],
                    tanh_bf[:, k, :],
                    start=(m == 0),
                    stop=(m == STRIP - 1),
                    tile_position=(0, g * STRIP),
                    skip_group_check=True,
                )
            j0 += bs

        # ---- epilogue for this block (rows are permuted) ----
        last = blk == n_blk - 1
        ctx_ps = ctx_ps_pool.tile([P, P + 1], F32)
        escore = esc_pool.tile([P, TE], BF16, tag="escore")
        tps = t_ps_pool.tile([P, n_te_chunk, P], BF16)
        escT = esc_pool.tile([P, n_te_chunk, P], BF16, tag="escT")
        if not last:
            # off the critical path: one big exp, then transposes
            nc.scalar.activation(
                out=escore, in_=score_ps, func=mybir.ActivationFunctionType.Exp
            )
            for c in range(n_te_chunk):
                nc.tensor.transpose(
                    tps[:, c, :], escore[:, c * P : (c + 1) * P], ident_bf
                )
            nc.vector.tensor_copy(escT, tps)
            for c in range(n_te_chunk):
                nc.tensor.matmul(
                    ctx_ps, escT[:, c, :], enc_ones[:, c, :],
                    start=(c == 0), stop=(c == n_te_chunk - 1),
                )
        else:
            # tail-latency critical: pipeline exp/transpose/copy/matmul per chunk
            for c in range(n_te_chunk):
                nc.scalar.activation(
                    out=escore[:, c * P : (c + 1) * P],
                    in_=score_ps[:, c * P : (c + 1) * P],
                    func=mybir.ActivationFunctionType.Exp,
                )
                nc.tensor.transpose(
                    tps[:, c, :], escore[:, c * P : (c + 1) * P], ident_bf
                )
                nc.vector.tensor_copy(escT[:, c, :], tps[:, c, :])
                nc.tensor.matmul(
                    ctx_ps, escT[:, c, :], enc_ones[:, c, :],
                    start=(c == 0), stop=(c == n_te_chunk - 1),
                )
        recip = out_pool.tile([P, 1], F32, tag="recip")
        nc.vector.reciprocal(recip, ctx_ps[:, P : P + 1])
        ctx_sb = out_pool.tile([P, P], F32, tag="ctx_sb")
        nc.vector.tensor_scalar_mul(out=ctx_sb, in0=ctx_ps[:, 0:P], scalar1=recip)

        # un-permute rows: PSUM partition 32g+m holds td 4m+g
        blk_rows = out_ap[blk * P : (blk + 1) * P, :].rearrange(
            "(m four) d -> four m d", four=N_STRIP
        )
        for g in range(N_STRIP):
            nc.sync.dma_start(
                out=blk_rows[g], in_=ctx_sb[g * STRIP : (g + 1) * STRIP, :]
            )


def build_program(td_n=TD_N):
    nc = bacc.Bacc("TRN2", target_bir_lowering=False, debug=False)
    n_te_chunk = TE // P
    decT = nc.dram_tensor("decT", [P, td_n], F32, kind="ExternalInput").ap()
    encT = nc.dram_tensor("encT", [P, TE], BF16, kind="ExternalInput").ap()
    enc_ones = nc.dram_tensor(
        "enc_ones", [P, n_te_chunk, P + 1], BF16, kind="ExternalInput"
    ).ap()
    v_bank = nc.dram_tensor("v_bank", [P, STRIP, STRIP], BF16, kind="ExternalInput").ap()
    w = nc.dram_tensor("w", [D, D], F32, kind="ExternalInput").ap()
    ident = nc.dram_tensor("ident", [P, P], BF16, kind="ExternalInput").ap()
    out = nc.dram_tensor("ctx_out", [td_n, D], F32, kind="ExternalOutput").ap()
    with tile.TileContext(nc) as tc, ExitStack() as ctx:
        _build_body(ctx, tc, out, decT, encT, enc_ones, v_bank, w, ident, td_n)
    nc.compile()
    return nc


def _prep_core_inputs(dec_slice, enc_b, w, v, bf16):
    """Host-side layout marshalling for one core (no reference FLOPs)."""
    n_te_chunk = TE // P
    decT = np.ascontiguousarray(dec_slice.T)                       # [d, td]
    encT = np.ascontiguousarray(enc_b.T).astype(bf16)              # [d, te]
    enc_ones = np.ones((P, n_te_chunk, P + 1), dtype=np.float32)
    # enc_ones[p, c, 0:128] = enc[c*128 + p, :]
    enc_ones[:, :, :P] = enc_b.reshape(n_te_chunk, P, D).transpose(1, 0, 2)
    v_bank = np.zeros((P, STRIP, STRIP), dtype=np.float32)
    idx = np.arange(STRIP)
    v_bank[:, idx, idx] = v[:, 0:1] * np.ones((P, STRIP), dtype=np.float32)
    ident = np.eye(P, dtype=np.float32)
    return {
        "decT": decT,
        "encT": encT,
        "enc_ones": enc_ones.astype(bf16),
        "v_bank": v_bank.astype(bf16),
        "w": np.ascontiguousarray(w),
        "ident": ident.astype(bf16),
    }


_CACHED_NC = None


def _run(inputs, trace=False):
    global _CACHED_NC
    if _CACHED_NC is None:
        _CACHED_NC = build_program()
    nc = _CACHED_NC
    bf16 = mybir.dt.np(BF16)

    dec = np.ascontiguousarray(inputs["decoder_outputs"], dtype=np.float32)
    enc = np.ascontiguousarray(inputs["encoder_outputs"], dtype=np.float32)
    w = np.ascontiguousarray(inputs["W"], dtype=np.float32)
    v = np.ascontiguousarray(inputs["V"], dtype=np.float32)

    in_maps = []
    for c in range(N_CORES):
        b, h = divmod(c, 2)
        in_maps.append(
            _prep_core_inputs(dec[b, h * TD_N : (h + 1) * TD_N], enc[b], w, v, bf16)
        )
    res = run_bass_kernel_spmd(nc, in_maps, core_ids=list(range(N_CORES)), trace=trace)
    out = np.zeros((B, TD, D), dtype=np.float32)
    for c in range(N_CORES):
        b, h = divmod(c, 2)
        out[b, h * TD_N : (h + 1) * TD_N] = res.results[c]["ctx_out"]
    return out, res


def kernel(**inputs):
    out, _ = _run(inputs, trace=False)
    return out


if __name__ == "__main__":
    rng = np.random.default_rng(0)
    inputs = {
        "decoder_outputs": rng.standard_normal((B, TD, D)).astype(np.float32),
        "encoder_outputs": rng.standard_normal((B, TE, D)).astype(np.float32),
        "W": (rng.uniform(-0.15, 0.15, (D, D))).astype(np.float32),
        "V": (rng.uniform(-0.21, 0.21, (D, 1))).astype(np.float32),
    }
    out = kernel(**inputs)
    print("ran, output shape", out.shape)


# revision 18
# speedup vs baseline: 1.0505x; 1.0006x over previous
"""Bahdanau additive-attention kernel for Trainium2, SPMD over 8 NeuronCores.

Reference computation (per batch b):
    dec_t  = dec @ W                                  [TD, D]
    score  = sum_d V[d] * tanh(dec_t[td,d] + enc[te,d])   [TD, TE]
    attn   = softmax(score, axis=te)
    ctx    = attn @ enc                               [TD, D]

Sharding: B=4, 8 cores -> core c handles batch b = c//2, td half h = c%2
(256 td rows each); enc/W replicated per batch. Host side does layout
marshalling only (transposes / dtype casts / placing V on diagonals);
all FLOPs of the reference computation run on device.

Per-core pipeline (sizes hardcoded: TD_N=256 td rows, TE=512, D=128):
 - PE:  dec_tT[e, td] = W.T-contracted matmul on the device.
 - DVE: tadd[d, te] = encT_bf[d, te] + dec_tT[d, td] via tensor_scalar_add,
   bf16 in/out -> 4x DVE mode (~345 ns per td row).
 - ACT: tanh over big batched instructions (bf16). This is the roofline
   engine: 16.8M elems / 128 lanes / 1.2 GHz ~ 110 us busy.
 - PE:  score row for td via accumulating matmul, lhsT = [128, 32] variant
   with V embedded in column m (v_bank). Four matmuls targeting the four
   32-partition column strips of the PSUM score tile run concurrently
   (tile_position col tiling); permutation td = 4*m + g lives at PSUM
   partition 32*g + m and is undone by the output DMA.
 - softmax without max-subtraction (|score| <= ||V||_1 ~ 14, exp safe):
   ACT exp(PSUM)->bf16, PE transpose, context matmul against [enc | ones]
   giving unnormalized context + denominator, then DVE reciprocal+scale.
"""

from contextlib import ExitStack

import numpy as np

import concourse.bacc as bacc
import concourse.tile as tile
from concourse import mybir
from concourse.bass_utils import run_bass_kernel_spmd

F32 = mybir.dt.float32
BF16 = mybir.dt.bfloat16

B, TD, TE, D = 4, 512, 512, 128
N_CORES = 8
TD_N = (B * TD) // N_CORES          # 256 td rows per core
P = 128
STRIP = 32                          # col-tiling strip width
N_STRIP = P // STRIP                # 4
# per-128-row-block ACT batch schedules: small first batches hide the DVE
# ramp; small last batches let the final score matmuls drain immediately.
BATCHES_FIRST = [4, 4, 8, 16, 32, 32, 32]
BATCHES_LAST = [32, 32, 32, 16, 8, 4, 4]


def _build_body(ctx, tc, out_ap, decT_ap, encT_ap, enc_ones_ap, v_bank_ap,
                w_ap, ident_ap, td_n):
    nc = tc.nc
    n_blk = td_n // P
    n_te_chunk = TE // P

    consts = ctx.enter_context(tc.tile_pool(name="consts", bufs=1))
    setup_ps = ctx.enter_context(tc.tile_pool(name="setup_ps", bufs=1, space="PSUM"))
    tadd_pool = ctx.enter_context(tc.tile_pool(name="tadd", bufs=3))
    score_ps_pool = ctx.enter_context(tc.tile_pool(name="score_ps", bufs=2, space="PSUM"))
    t_ps_pool = ctx.enter_context(tc.tile_pool(name="t_ps", bufs=2, space="PSUM"))
    ctx_ps_pool = ctx.enter_context(tc.tile_pool(name="ctx_ps", bufs=2, space="PSUM"))
    esc_pool = ctx.enter_context(tc.tile_pool(name="esc", bufs=2))
    out_pool = ctx.enter_context(tc.tile_pool(name="outp", bufs=2))

    # ---- inputs (pre-marshalled on host); chunked so the first adds start
    # as early as possible ----
    head_n = BATCHES_FIRST[0]
    decT_head = consts.tile([P, head_n], F32)     # [d, first tds] fast path
    nc.sync.dma_start(out=decT_head, in_=decT_ap[:, 0:head_n])
    w_sb = consts.tile([P, P], F32)               # [d, e]
    nc.scalar.dma_start(out=w_sb, in_=w_ap)
    encT_bf = consts.tile([P, TE], BF16)          # [d, te]
    nc.sync.dma_start(out=encT_bf, in_=encT_ap)
    decT = consts.tile([P, td_n], F32)            # [d, td]
    nc.sync.dma_start(out=decT[:, 0:P], in_=decT_ap[:, 0:P])
    for c in range(1, n_blk):
        nc.scalar.dma_start(
            out=decT[:, c * P : (c + 1) * P], in_=decT_ap[:, c * P : (c + 1) * P]
        )
    v_bank = consts.tile([P, STRIP, STRIP], BF16)  # variant m: V in col m
    nc.gpsimd.dma_start(out=v_bank, in_=v_bank_ap)
    enc_ones = consts.tile([P, n_te_chunk, P + 1], BF16)   # [te | 1.0]
    nc.gpsimd.dma_start(out=enc_ones, in_=enc_ones_ap)
    ident_bf = consts.tile([P, P], BF16)
    nc.gpsimd.dma_start(out=ident_bf, in_=ident_ap)

    # dec_tT[e, td] = sum_d W[d,e] * decT[d, td].
    # Fast path: the first head_n columns via a tiny matmul so the DVE adds
    # (and thus ACT) start as early as possible; then one tile per block.
    mp_h = setup_ps.tile([P, head_n], F32, tag="mp_head")
    nc.tensor.matmul(mp_h, w_sb, decT_head, start=True, stop=True)
    dec_tT_head = consts.tile([P, head_n], F32)
    nc.vector.tensor_copy(dec_tT_head, mp_h)
    dec_tT = []
    for c in range(n_blk):
        mp = setup_ps.tile([P, P], F32)
        nc.tensor.matmul(mp, w_sb, decT[:, c * P : (c + 1) * P], start=True, stop=True)
        t = consts.tile([P, P], F32, tag=f"dec_tT{c}")
        nc.vector.tensor_copy(t, mp)
        dec_tT.append(t)

    # ---- main loop ----
    for blk in range(n_blk):
        batches = BATCHES_FIRST if blk == 0 else BATCHES_LAST
        assert sum(batches) == P
        score_ps = score_ps_pool.tile([P, TE], F32)
        j0 = 0
        for bs in batches:
            tadd = tadd_pool.tile([P, bs, TE], BF16, tag="tadd")
            for k in range(bs):
                if blk == 0 and j0 + k < head_n:
                    scal = dec_tT_head[:, j0 + k : j0 + k + 1]
                else:
                    scal = dec_tT[blk][:, j0 + k : j0 + k + 1]
                nc.vector.tensor_scalar_add(
                    out=tadd[:, k, :], in0=encT_bf, scalar1=scal,
                )
            # tanh in place: halves SBUF traffic and tile count
            tanh_bf = tadd
            nc.scalar.activation(
                out=tanh_bf, in_=tadd, func=mybir.ActivationFunctionType.Tanh
            )
            # V-reduce: td-in-block j = 4*m + g -> PSUM partition 32*g + m.
            # Quad of col-strip matmuls per m runs concurrently on PE.
            for k in range(bs):
                j = j0 + k
                m, g = divmod(j, N_STRIP)
                nc.tensor.matmul(
                    score_ps[g * STRIP : (g + 1) * STRIP, :],
                    v_bank[:, m, :],
                    tanh_bf[:, k, :],
                    start=(m == 0),
                    stop=(m == STRIP - 1),
                    tile_position=(0, g * STRIP),
                    skip_group_check=True,
                )
            j0 += bs

        # ---- epilogue for this block (rows are permuted) ----
        last = blk == n_blk - 1
        ctx_ps = ctx_ps_pool.tile([P, P + 1], F32)
        escore = esc_pool.tile([P, TE], BF16, tag="escore")
        tps = t_ps_pool.tile([P, n_te_chunk, P], BF16)
        escT = esc_pool.tile([P, n_te_chunk, P], BF16, tag="escT")
        if not last:
            # off the critical path: one big exp, then transposes
            nc.scalar.activation(
                out=escore, in_=score_ps, func=mybir.ActivationFunctionType.Exp
            )
            for c in range(n_te_chunk):
                nc.tensor.transpose(
                    tps[:, c, :], escore[:, c * P : (c + 1) * P], ident_bf
                )
            nc.vector.tensor_copy(escT, tps)
            for c in range(n_te_chunk):
                nc.tensor.matmul(
                    ctx_ps, escT[:, c, :], enc_ones[:, c, :],
                    start=(c == 0), stop=(c == n_te_chunk - 1),
                )
        else:
            # tail-latency critical: pipeline exp/transpose/copy/matmul per chunk
            for c in range(n_te_chunk):
                nc.scalar.activation(
                    out=escore[:, c * P : (c + 1) * P],
                    in_=score_ps[:, c * P : (c + 1) * P],
                    func=mybir.ActivationFunctionType.Exp,
                )
                nc.tensor.transpose(
                    tps[:, c, :], escore[:, c * P : (c + 1) * P], ident_bf
                )
                nc.vector.tensor_copy(escT[:, c, :], tps[:, c, :])
                nc.tensor.matmul(
                    ctx_ps, escT[:, c, :], enc_ones[:, c, :],
                    start=(c == 0), stop=(c == n_te_chunk - 1),
                )
        recip = out_pool.tile([P, 1], F32, tag="recip")
        nc.vector.reciprocal(recip, ctx_ps[:, P : P + 1])
        ctx_sb = out_pool.tile([P, P], F32, tag="ctx_sb")
        nc.vector.tensor_scalar_mul(out=ctx_sb, in0=ctx_ps[:, 0:P], scalar1=recip)

        # un-permute rows: PSUM partition 32g+m holds td 4m+g
        blk_rows = out_ap[blk * P : (blk + 1) * P, :].rearrange(
            "(m four) d -> four m d", four=N_STRIP
        )
        for g in range(N_STRIP):
            eng = nc.sync if g % 2 == 0 else nc.scalar
            eng.dma_start(
                out=blk_rows[g], in_=ctx_sb[g * STRIP : (g + 1) * STRIP, :]
            )


def build_program(td_n=TD_N):
    nc = bacc.Bacc("TRN2", target_bir_lowering=False, debug=False)
    n_te_chunk = TE // P
    decT = nc.dram_tensor("decT", [P, td_n], F32, kind="ExternalInput").ap()
    encT = nc.dram_tensor("encT", [P, TE], BF16, kind="ExternalInput").ap()
    enc_ones = nc.dram_tensor(
        "enc_ones", [P, n_te_chunk, P + 1], BF16, kind="ExternalInput"
    ).ap()
    v_bank = nc.dram_tensor("v_bank", [P, STRIP, STRIP], BF16, kind="ExternalInput").ap()
    w = nc.dram_tensor("w", [D, D], F32, kind="ExternalInput").ap()
    ident = nc.dram_tensor("ident", [P, P], BF16, kind="ExternalInput").ap()
    out = nc.dram_tensor("ctx_out", [td_n, D], F32, kind="ExternalOutput").ap()
    with tile.TileContext(nc) as tc, ExitStack() as ctx:
        _build_body(ctx, tc, out, decT, encT, enc_ones, v_bank, w, ident, td_n)
    nc.compile()
    return nc


def _prep_core_inputs(dec_slice, enc_b, w, v, bf16):
    """Host-side layout marshalling for one core (no reference FLOPs)."""
    n_te_chunk = TE // P
    decT = np.ascontiguousarray(dec_slice.T)                       # [d, td]
    encT = np.ascontiguousarray(enc_b.T).astype(bf16)              # [d, te]
    enc_ones = np.ones((P, n_te_chunk, P + 1), dtype=np.float32)
    # enc_ones[p, c, 0:128] = enc[c*128 + p, :]
    enc_ones[:, :, :P] = enc_b.reshape(n_te_chunk, P, D).transpose(1, 0, 2)
    v_bank = np.zeros((P, STRIP, STRIP), dtype=np.float32)
    idx = np.arange(STRIP)
    v_bank[:, idx, idx] = v[:, 0:1] * np.ones((P, STRIP), dtype=np.float32)
    ident = np.eye(P, dtype=np.float32)
    return {
        "decT": decT,
        "encT": encT,
        "enc_ones": enc_ones.astype(bf16),
        "v_bank": v_bank.astype(bf16),
        "w": np.ascontiguousarray(w),
        "ident": ident.astype(bf16),
    }


_CACHED_NC = None


def _run(inputs, trace=False):
    global _CACHED_NC
    if _CACHED_NC is None:
        _CACHED_NC = build_program()
    nc = _CACHED_NC
    bf16 = mybir.dt.np(BF16)

    dec = np.ascontiguousarray(inputs["decoder_outputs"], dtype=np.float32)
    enc = np.ascontiguousarray(inputs["encoder_outputs"], dtype=np.float32)
    w = np.ascontiguousarray(inputs["W"], dtype=np.float32)
    v = np.ascontiguousarray(inputs["V"], dtype=np.float32)

    in_maps = []
    for c in range(N_CORES):
        b, h = divmod(c, 2)
        in_maps.append(
            _prep_core_inputs(dec[b, h * TD_N : (h + 1) * TD_N], enc[b], w, v, bf16)
        )
    res = run_bass_kernel_spmd(nc, in_maps, core_ids=list(range(N_CORES)), trace=trace)
    out = np.zeros((B, TD, D), dtype=np.float32)
    for c in range(N_CORES):
        b, h = divmod(c, 2)
        out[b, h * TD_N : (h + 1) * TD_N] = res.results[c]["ctx_out"]
    return out, res


def kernel(**inputs):
    out, _ = _run(inputs, trace=False)
    return out


if __name__ == "__main__":
    rng = np.random.default_rng(0)
    inputs = {
        "decoder_outputs": rng.standard_normal((B, TD, D)).astype(np.float32),
        "encoder_outputs": rng.standard_normal((B, TE, D)).astype(np.float32),
        "W": (rng.uniform(-0.15, 0.15, (D, D))).astype(np.float32),
        "V": (rng.uniform(-0.21, 0.21, (D, 1))).astype(np.float32),
    }
    out = kernel(**inputs)
    print("ran, output shape", out.shape)
